# revision 1
# baseline (speedup 1.0000x reference)
"""MicroTransformer forward pass on 8 trn2 NeuronCores.

Sharding: DP2 (batch) x CP4 (strided context parallel).
Core c = (b, p), b = c // 4, p = c % 4, owns tokens at global positions
p, p+4, p+8, ... of batch b  (T = S/4 tokens per core).

Per layer each core computes qkv for its own tokens + RoPE; K and
(transposed, ones-augmented) V shards are AllGathered in bf16 within the
4-core batch group; attention runs over the gathered shard-major K/V
with per-shard causal masks (softmax is permutation-invariant so
shard-major KV order is fine); out-proj and SwiGLU FFN are token-local.
LM head: own tokens x full vocab.  Norm weights are folded into the
following weight matrix on the host; weights arrive pre-transposed so
the contraction dim lies on SBUF partitions.

On-chip layout: activations are feature-major [feat, tok]; matmul
outputs [out_feat, tok] feed the next matmul's moving operand directly.
Partition-dim reductions (RMSNorm sum, softmax denom) use ones-vector /
ones-column matmuls on the PE.  Scores are computed transposed
([tk, tq]) so the exp output feeds the AV matmul without a transpose.
Big matmuls run in float32r (full PE rate at N>=256); QK^T / AV run in
bf16 with fp32 PSUM accumulation.
"""

import numpy as np

try:
    import concourse.bass as bass  # noqa: F401
except ImportError:
    import sys

    sys.path.insert(0, "/opt/trn_rl_repo")
    import concourse.bass as bass  # noqa: F401

import concourse.bacc as bacc
import concourse.mybir as mybir
import concourse.tile as tile
from concourse.bass_utils import run_bass_kernel_spmd
from concourse.masks import make_identity

F32 = mybir.dt.float32
F32R = mybir.dt.float32r
BF16 = mybir.dt.bfloat16
AF = mybir.ActivationFunctionType

NEG = -1e30

CFG_FULL = dict(V=32000, D=1024, L=8, F=4096, S=2048, H=16, HD=64, LM_OC=5)


class Ctx:
    def __init__(self, cfg):
        self.__dict__.update(cfg)
        self.B = 2
        self.T = self.S // 4        # tokens per core
        self.DT = self.D // 128     # 128-row feature tiles of x/h
        self.FT = self.F // 128
        self.NTK = self.T // 128    # tk chunks per shard (= tq blocks)
        assert self.T % 128 == 0 and self.D % 128 == 0 and self.HD == 64
        self.VW = (self.H // 2) * 130  # per head pair: v_even|ones|v_odd|pad
        self.KVK = self.D * self.T            # k-region elems in kv dump
        self.KVV = self.T * self.VW           # v-region elems
        self.KVE = self.KVK + self.KVV


def _t(pool, shape, dtype, tag):
    return pool.tile(shape, dtype, tag=tag, name=tag)


def _r(ap):
    return ap.bitcast(F32R)


def build_nc(cfg, n_cores=8):
    c = Ctx(cfg)
    nc = bacc.Bacc("TRN2", target_bir_lowering=False, debug=False,
                   num_devices=n_cores)
    if n_cores == 8:
        groups = [[0, 1, 2, 3], [4, 5, 6, 7]]
    else:
        groups = [list(range(n_cores))]

    D, T, L, F, V = c.D, c.T, c.L, c.F, c.V

    io = {}
    def inp(name, shape):
        io[name] = nc.dram_tensor(name, shape, F32, kind="ExternalInput").ap()
    inp("x0", [D, T])
    inp("qkvT", [L, D, 3 * D])
    inp("owT", [L, D, D])
    inp("w1T", [L, D, F])
    inp("w3T", [L, D, F])
    inp("w2T", [L, F, D])
    inp("embT", [D, V])
    inp("cosq", [128, T])
    inp("sinq", [128, T])
    inp("ropeP", [128, 128])
    inp("maskA", [4, 128, 128])
    io["logits"] = nc.dram_tensor("logits", [V, T], F32,
                                  kind="ExternalOutput").ap()

    with tile.TileContext(nc) as tc:
        _emit(tc, c, groups, io)
    nc.compile()
    return nc


def _emit(tc, c, groups, io):
    nc = tc.nc
    D, T, L, F, V, H = c.D, c.T, c.L, c.F, c.V, c.H
    DT, FT, NTK, VW = c.DT, c.FT, c.NTK, c.VW
    FQ = 4 if FT % 4 == 0 else FT   # f-chunks per FFN sub-phase (<=4 -> 512 cols)
    scale = c.HD ** -0.5
    WCOL = 512                       # weight tile width (columns)

    ctx_pools = []
    def pool(**kw):
        p = tc.tile_pool(**kw)
        v = p.__enter__()
        ctx_pools.append(p)
        return v

    perst = pool(name="perst", bufs=1)
    wpool = pool(name="wpool", bufs=1)      # streamed weight tiles [128, 512]
    apool = pool(name="apool", bufs=1)      # per-layer activations (by tag)
    spool1 = pool(name="spool1", bufs=1)    # norm / denom staging
    spool2 = pool(name="spool2", bufs=2)    # rotating staging tiles
    kpool = pool(name="kpool", bufs=2)      # gathered K tiles, per shard
    vgpool = pool(name="vgpool", bufs=2)    # gathered V pair tiles
    epool = pool(name="epool", bufs=17)     # exp tiles
    gpool = pool(name="gpool", bufs=1)      # gate tiles (per f-chunk in quarter)
    gupool = pool(name="gupool", bufs=1)    # gate*up tiles (quarter-local)
    ps = pool(name="ps", bufs=3, space="PSUM")
    ps_s = pool(name="ps_s", bufs=3, space="PSUM")
    ps_o = pool(name="ps_o", bufs=2, space="PSUM")
    dram = pool(name="dram", bufs=2, space="DRAM")

    # ---------- persistent tiles ----------
    xt = [_t(perst, [128, T], F32, f"x{i}") for i in range(DT)]
    cos_t = _t(perst, [128, T], F32, "cos")
    sin_t = _t(perst, [128, T], F32, "sin")
    ropeP_t = _t(perst, [128, 128], F32, "ropeP")
    ident_t = _t(perst, [128, 128], F32, "ident")
    mask_t = [_t(perst, [128, 128], F32, f"mask{s}") for s in range(4)]
    ones_t = _t(perst, [128, 1], F32, "ones")
    eps_t = _t(perst, [1, 1], F32, "eps")
    nc.gpsimd.memset(eps_t[:], 1e-6)

    nc.sync.dma_start(cos_t[:], io["cosq"][:])
    nc.sync.dma_start(sin_t[:], io["sinq"][:])
    nc.sync.dma_start(_r(ropeP_t[:]), _r(io["ropeP"][:]))
    for s in range(4):
        nc.sync.dma_start(mask_t[s][:], io["maskA"][s])
    ones_raw = _t(perst, [128, 1], F32, "ones_raw")
    nc.gpsimd.memset(ones_raw[:], 1.0)
    nc.vector.tensor_copy(_r(ones_t[:]), ones_raw[:])
    make_identity(nc, ident_t[:])
    for i in range(DT):
        nc.sync.dma_start(xt[i][:], io["x0"][i * 128:(i + 1) * 128, :])

    def load_w(dram_ap, r0, c0, rows=128, cols=WCOL):
        """Stream a [rows, cols] weight tile; slots shared via tag family."""
        t = _t(wpool, [128, WCOL], F32, f"w{(r0 // 128) % 8}")
        nc.sync.dma_start(_r(t[:rows, :cols]),
                          _r(dram_ap[r0:r0 + rows, c0:c0 + cols]))
        return t

    def rmsnorm():
        """h = x * rsqrt(mean(x^2) + eps). Returns h tiles (feature-major)."""
        ssum = _t(ps, [128, T], F32, "mm")
        for i in range(DT):
            sqt = _t(spool1, [128, T], F32, "nsq")
            nc.vector.tensor_mul(_r(sqt[:]), xt[i][:], xt[i][:])
            nc.tensor.matmul(ssum[0:1, :], _r(ones_t[:]), _r(sqt[:]),
                             start=(i == 0), stop=(i == DT - 1))
        srt = _t(spool1, [1, T], F32, "nsrt")
        nc.scalar.activation(srt[:], ssum[0:1, :], AF.Sqrt,
                             bias=eps_t[:], scale=1.0 / D)
        rs = _t(spool1, [1, T], F32, "nrs")
        nc.vector.reciprocal(rs[:], srt[:])
        rb = _t(spool1, [128, T], F32, "nrb")
        nc.gpsimd.partition_broadcast(rb[:], rs[:])
        hts = []
        for i in range(DT):
            h = _t(apool, [128, T], F32, f"h{i}")
            nc.vector.tensor_mul(_r(h[:]), xt[i][:], rb[:])
            hts.append(h)
        return hts

    for layer in range(L):
        # ================= attention =================
        hts = rmsnorm()

        qp = [_t(apool, [128, T], BF16, f"qp{i}") for i in range(DT)]
        vT = [_t(apool, [128, VW], BF16, f"vT{b}") for b in range(NTK)]
        for b in range(NTK):
            nc.gpsimd.memset(vT[b][:], 1.0)

        kv_shard = _t(dram, [c.KVE], BF16, "kv_shard")
        kv_all = dram.tile([len(groups[0]) * c.KVE], BF16, tag="kv_all",
                           name="kv_all")

        # --- q and k (RoPE); k dumped to kv_shard, q kept ---
        for which in (0, 1):  # 0 = q, 1 = k
            wsec = []
            for k in range(DT):
                wt0 = load_w(io["qkvT"][layer], k * 128, which * D)
                wt1 = load_w(io["qkvT"][layer], k * 128, which * D + WCOL) \
                    if D > WCOL else None
                wsec.append((wt0, wt1))
            for i in range(DT):
                pm = _t(ps, [128, T], F32, "mm")
                col = i * 128
                for k in range(DT):
                    wt = wsec[k][0] if col < WCOL else wsec[k][1]
                    cc = col % WCOL
                    nc.tensor.matmul(pm[:], _r(wt[:, cc:cc + 128]),
                                     _r(hts[k][:]),
                                     start=(k == 0), stop=(k == DT - 1))
                sb = _t(spool2, [128, T], F32, "tmp")
                nc.vector.tensor_copy(_r(sb[:]), pm[:])
                rot = _t(ps, [128, T], F32, "mm")
                nc.tensor.matmul(rot[:], _r(ropeP_t[:]), _r(sb[:]),
                                 start=True, stop=True)
                t1 = _t(spool2, [128, T], F32, "rope1")
                nc.vector.tensor_mul(t1[:], sb[:], cos_t[:])
                t2 = _t(spool2, [128, T], F32, "rope2")
                nc.vector.tensor_mul(t2[:], rot[:], sin_t[:])
                if which == 0:
                    nc.vector.tensor_add(qp[i][:], t1[:], t2[:])
                else:
                    kb = _t(spool2, [128, T], BF16, "kb")
                    nc.vector.tensor_add(kb[:], t1[:], t2[:])
                    dst = kv_shard[i * 128 * T:(i + 1) * 128 * T]
                    nc.sync.dma_start(dst.rearrange("(p t) -> p t", p=128),
                                      kb[:])

        # --- v: compute, transpose into v_aug layout, dump ---
        wsec = []
        for k in range(DT):
            wt0 = load_w(io["qkvT"][layer], k * 128, 2 * D)
            wt1 = load_w(io["qkvT"][layer], k * 128, 2 * D + WCOL) \
                if D > WCOL else None
            wsec.append((wt0, wt1))
        for i in range(DT):
            pm = _t(ps, [128, T], F32, "mm")
            col = i * 128
            for k in range(DT):
                wt = wsec[k][0] if col < WCOL else wsec[k][1]
                cc = col % WCOL
                nc.tensor.matmul(pm[:], _r(wt[:, cc:cc + 128]), _r(hts[k][:]),
                                 start=(k == 0), stop=(k == DT - 1))
            vsb = _t(spool2, [128, T], F32, "tmp")
            nc.vector.tensor_copy(vsb[:], pm[:])
            for b in range(NTK):
                pt = _t(ps_s, [128, 128], F32, "st")
                nc.tensor.transpose(pt[:], vsb[:, b * 128:(b + 1) * 128],
                                    ident_t[:])
                # pair block i: cols [130i,130i+64)=v_even, 130i+64=ones,
                # [130i+65,130i+129)=v_odd, 130i+129=pad
                nc.vector.tensor_copy(vT[b][:, 130 * i:130 * i + 64],
                                      pt[:, 0:64])
                nc.vector.tensor_copy(vT[b][:, 130 * i + 65:130 * i + 129],
                                      pt[:, 64:128])
        for b in range(NTK):
            off = c.KVK + b * 128 * VW
            dst = kv_shard[off:off + 128 * VW]
            nc.sync.dma_start(dst.rearrange("(p t) -> p t", p=128), vT[b][:])

        # --- AllGather K/V within the 4-core group ---
        nc.gpsimd.collective_compute(
            "AllGather", mybir.AluOpType.bypass, replica_groups=groups,
            ins=[kv_shard.opt()], outs=[kv_all.opt()])

        # --- attention: head pairs share a gathered-K feature tile ---
        for i in range(DT):
            kg = []
            for s in range(4):
                t = _t(kpool, [128, T], BF16, f"kg{s}")
                off = s * c.KVE + i * 128 * T
                nc.sync.dma_start(
                    t[:], kv_all[off:off + 128 * T].rearrange(
                        "(p t) -> p t", p=128))
                kg.append(t)
            vgp = {}
            for s in range(4):
                for b in range(NTK):
                    t = _t(vgpool, [128, 130], BF16, f"vg{s}_{b}")
                    off = s * c.KVE + c.KVK + b * 128 * VW
                    full = kv_all[off:off + 128 * VW].rearrange(
                        "(p t) -> p t", p=128)
                    nc.sync.dma_start(t[:], full[:, 130 * i:130 * i + 130])
                    vgp[(s, b)] = t
            aop = _t(spool2, [128, T], F32, "aop")
            for hh in range(2):
                r0 = hh * 64
                ech = {}
                for s in range(4):
                    for ck in range(NTK):
                        q0 = ck * 128
                        st = _t(ps_s, [128, T], F32, "st")
                        kl = kg[s][r0:r0 + 64, q0:q0 + 128]
                        nc.tensor.matmul(st[:, q0:q0 + 128], kl,
                                         qp[i][r0:r0 + 64, q0:q0 + 128],
                                         start=True, stop=True)
                        if q0 + 128 < T:
                            nc.tensor.matmul(st[:, q0 + 128:T], kl,
                                             qp[i][r0:r0 + 64, q0 + 128:T],
                                             start=True, stop=True)
                        nc.vector.tensor_add(st[:, q0:q0 + 128],
                                             st[:, q0:q0 + 128], mask_t[s][:])
                        e = _t(epool, [128, T], BF16, "e")
                        nc.scalar.activation(e[:, q0:T], st[:, q0:T], AF.Exp,
                                             scale=scale)
                        ech[(s, ck)] = e
                o_ps = _t(ps_o, [128, T], F32, "oaug")
                for b in range(NTK):
                    q0 = b * 128
                    n = 0
                    tot = 4 * (b + 1)
                    for s in range(4):
                        for ck in range(b + 1):
                            if hh == 0:
                                # [v_even | ones]: rows 0:64 = o, row 64 = den
                                nc.tensor.matmul(
                                    o_ps[0:65, q0:q0 + 128],
                                    vgp[(s, ck)][:, 0:65],
                                    ech[(s, ck)][:, q0:q0 + 128],
                                    start=(n == 0), stop=(n == tot - 1))
                            else:
                                # v_odd -> rows 64:128; ones col -> row 0
                                nc.tensor.matmul(
                                    o_ps[64:128, q0:q0 + 128],
                                    vgp[(s, ck)][:, 65:129],
                                    ech[(s, ck)][:, q0:q0 + 128],
                                    start=(n == 0), stop=(n == tot - 1))
                                nc.tensor.matmul(
                                    o_ps[0:1, q0:q0 + 128],
                                    vgp[(s, ck)][:, 64:65],
                                    ech[(s, ck)][:, q0:q0 + 128],
                                    start=(n == 0), stop=(n == tot - 1))
                            n += 1
                den = _t(spool1, [1, T], F32, "den")
                if hh == 0:
                    # denom at partition 64: copy out at 64, DMA down to 0
                    den128 = _t(spool1, [128, T], F32, "den128")
                    nc.vector.tensor_copy(den128[64:65, :], o_ps[64:65, :])
                    nc.sync.dma_start(den[:], den128[64:65, :])
                else:
                    nc.vector.tensor_copy(den[:], o_ps[0:1, :])
                rec = _t(spool1, [1, T], F32, "rec")
                nc.vector.reciprocal(rec[:], den[:])
                rb128 = _t(spool1, [128, T], F32, "recb")
                nc.gpsimd.partition_broadcast(rb128[:], rec[:])
                if hh == 0:
                    nc.vector.tensor_mul(_r(aop[0:64, :]), o_ps[0:64, :],
                                         rb128[0:64, :])
                else:
                    nc.vector.tensor_mul(_r(aop[64:128, :]), o_ps[64:128, :],
                                         rb128[64:128, :])
            # out-projection contribution of this head pair, into residual
            wo0 = load_w(io["owT"][layer], i * 128, 0)
            wo1 = None
            if D > WCOL:
                wo1 = _t(wpool, [128, WCOL], F32, f"wb{i % 8}")
                nc.sync.dma_start(
                    _r(wo1[:]), _r(io["owT"][layer][i * 128:(i + 1) * 128,
                                                    WCOL:2 * WCOL]))
            for oc in range(DT):
                pm = _t(ps, [128, T], F32, "mm")
                col = oc * 128
                wt = wo0 if col < WCOL else wo1
                cc = col % WCOL
                nc.tensor.matmul(pm[:], _r(wt[:, cc:cc + 128]), _r(aop[:]),
                                 start=True, stop=True)
                nc.vector.tensor_add(xt[oc][:], xt[oc][:], pm[:])

        # ================= FFN (SwiGLU), quarter-fused =================
        hts = rmsnorm()
        for q0f in range(0, FT, FQ):
            f1 = min(q0f + FQ, FT)
            nf = f1 - q0f
            # gate = silu(w1 h)
            w1 = [load_w(io["w1T"][layer], k * 128, q0f * 128,
                         cols=min(WCOL, nf * 128)) for k in range(DT)]
            g = []
            for f in range(q0f, f1):
                pm = _t(ps, [128, T], F32, "mm")
                for k in range(DT):
                    cc = (f - q0f) * 128
                    nc.tensor.matmul(pm[:], _r(w1[k][:, cc:cc + 128]),
                                     _r(hts[k][:]),
                                     start=(k == 0), stop=(k == DT - 1))
                sg = _t(spool2, [128, T], F32, "sg")
                nc.scalar.activation(sg[:], pm[:], AF.Sigmoid)
                gt = _t(gpool, [128, T], F32, f"g{f - q0f}")
                nc.vector.tensor_mul(gt[:], sg[:], pm[:])
                g.append(gt)
            # up = w3 h ; gu = gate * up
            w3 = [load_w(io["w3T"][layer], k * 128, q0f * 128,
                         cols=min(WCOL, nf * 128)) for k in range(DT)]
            gu = []
            for f in range(q0f, f1):
                pm = _t(ps, [128, T], F32, "mm")
                for k in range(DT):
                    cc = (f - q0f) * 128
                    nc.tensor.matmul(pm[:], _r(w3[k][:, cc:cc + 128]),
                                     _r(hts[k][:]),
                                     start=(k == 0), stop=(k == DT - 1))
                gut = _t(gupool, [128, T], F32, f"gu{f - q0f}")
                nc.vector.tensor_mul(_r(gut[:]), g[f - q0f][:], pm[:])
                gu.append(gut)
            # partial w2 contribution for this quarter, accumulated into x
            w2 = []
            for f in range(q0f, f1):
                wt0 = _t(wpool, [128, WCOL], F32, f"w{(f - q0f) % 8}")
                nc.sync.dma_start(
                    _r(wt0[:]),
                    _r(io["w2T"][layer][f * 128:(f + 1) * 128, 0:WCOL]))
                wt1 = None
                if D > WCOL:
                    wt1 = _t(wpool, [128, WCOL], F32, f"wb{(f - q0f) % 8}")
                    nc.sync.dma_start(
                        _r(wt1[:]),
                        _r(io["w2T"][layer][f * 128:(f + 1) * 128,
                                            WCOL:2 * WCOL]))
                w2.append((wt0, wt1))
            for oc in range(DT):
                pm = _t(ps, [128, T], F32, "mm")
                col = oc * 128
                for f in range(nf):
                    wt = w2[f][0] if col < WCOL else w2[f][1]
                    cc = col % WCOL
                    nc.tensor.matmul(pm[:], _r(wt[:, cc:cc + 128]),
                                     _r(gu[f][:]),
                                     start=(f == 0), stop=(f == nf - 1))
                nc.vector.tensor_add(xt[oc][:], xt[oc][:], pm[:])

    # ================= final norm + LM head =================
    hts = rmsnorm()
    VB = c.LM_OC * 128
    assert V % VB == 0
    for vb in range(V // VB):
        we = [load_w(io["embT"], k * 128, vb * VB, cols=min(WCOL, VB))
              for k in range(DT)]
        we2 = None
        if VB > WCOL:
            we2 = [_t(wpool, [128, WCOL], F32, f"wb{k % 8}")
                   for k in range(DT)]
            for k in range(DT):
                nc.sync.dma_start(
                    _r(we2[k][:, :VB - WCOL]),
                    _r(io["embT"][k * 128:(k + 1) * 128,
                                  vb * VB + WCOL:(vb + 1) * VB]))
        for o in range(c.LM_OC):
            pm = _t(ps, [128, T], F32, "mm")
            col = o * 128
            for k in range(DT):
                wt = we[k] if col < WCOL else we2[k]
                cc = col % WCOL
                nc.tensor.matmul(pm[:], _r(wt[:, cc:cc + 128]), _r(hts[k][:]),
                                 start=(k == 0), stop=(k == DT - 1))
            sb = _t(spool2, [128, T], F32, "tmp")
            nc.vector.tensor_copy(sb[:], pm[:])
            nc.sync.dma_start(
                io["logits"][vb * VB + col: vb * VB + col + 128, :], sb[:])

    for p in reversed(ctx_pools):
        p.__exit__(None, None, None)


# ============================================================
# host side
# ============================================================

def host_prep(cfg, inputs, n_cores=8):
    c = Ctx(cfg)
    ids = np.asarray(inputs["input_ids"])
    emb = np.asarray(inputs["emb"], np.float32)
    anw = np.asarray(inputs["attn_norm_w"], np.float32)
    fnw = np.asarray(inputs["ffn_norm_w"], np.float32)
    lnw = np.asarray(inputs["final_norm_w"], np.float32)

    qkvT = np.ascontiguousarray(
        np.transpose(np.asarray(inputs["qkv_w"], np.float32), (0, 2, 1))
        * anw[:, :, None])
    owT = np.ascontiguousarray(
        np.transpose(np.asarray(inputs["out_w"], np.float32), (0, 2, 1)))
    w1T = np.ascontiguousarray(
        np.transpose(np.asarray(inputs["w1"], np.float32), (0, 2, 1))
        * fnw[:, :, None])
    w3T = np.ascontiguousarray(
        np.transpose(np.asarray(inputs["w3"], np.float32), (0, 2, 1))
        * fnw[:, :, None])
    w2T = np.ascontiguousarray(
        np.transpose(np.asarray(inputs["w2"], np.float32), (0, 2, 1)))
    embT = np.ascontiguousarray(emb.T * lnw[:, None])

    hd = c.HD
    inv = 1.0 / (10000.0 ** (np.arange(0, hd, 2, dtype=np.float32) / hd))

    P = np.zeros((128, 128), np.float32)
    for head in range(2):
        b = head * 64
        for m in range(32):
            P[b + m + 32, b + m] = -1.0   # rot[j] = -q[j+32], j < 32
            P[b + m, b + m + 32] = 1.0    # rot[j] =  q[j-32], j >= 32
    k_idx = np.arange(128)[:, None]
    j_idx = np.arange(128)[None, :]
    tri_incl = np.where(k_idx <= j_idx, 0.0, NEG).astype(np.float32)
    tri_strict = np.where(k_idx < j_idx, 0.0, NEG).astype(np.float32)

    in_maps = []
    for core in range(n_cores):
        b, p = core // 4, core % 4
        tok = np.asarray(ids[b, p::4], np.int64)
        x0 = np.ascontiguousarray(emb[tok].T)
        pos = np.arange(p, c.S, 4, dtype=np.float32)
        fr = pos[:, None] * inv[None, :]
        ang = np.concatenate([fr, fr], axis=1)          # [T, hd]
        cosq = np.ascontiguousarray(np.tile(np.cos(ang).T, (2, 1)))
        sinq = np.ascontiguousarray(np.tile(np.sin(ang).T, (2, 1)))
        maskA = np.ascontiguousarray(
            np.stack([tri_incl if s <= p else tri_strict for s in range(4)]))
        in_maps.append(dict(
            x0=x0, qkvT=qkvT, owT=owT, w1T=w1T, w3T=w3T, w2T=w2T, embT=embT,
            cosq=cosq, sinq=sinq, ropeP=P, maskA=maskA))
    return in_maps


def assemble(cfg, results):
    c = Ctx(cfg)
    out = np.empty((c.B, c.S, c.V), np.float32)
    for core in range(len(results)):
        b, p = core // 4, core % 4
        out[b, p::4, :] = results[core]["logits"].T
    return out


_NC_CACHE = {}


def kernel(**inputs):
    cfg = CFG_FULL
    if "full" not in _NC_CACHE:
        _NC_CACHE["full"] = build_nc(cfg)
    nc = _NC_CACHE["full"]
    in_maps = host_prep(cfg, inputs)
    res = run_bass_kernel_spmd(nc, in_maps, list(range(8)))
    return assemble(cfg, res.results)

